# revision 1
# baseline (speedup 1.0000x reference)
"""Trainium2 Bass kernel for CrossAttentionFusion.

Reference computation (shapes hardcoded):
  B=4, C=256, H=W=128, N=16384, CHUNK=2048, nchunks=8.
  q  = image_features  reshaped to (B, nchunks, CHUNK, C)
  kv = lidar_features  reshaped to (B, nchunks, CHUNK, C)
  per (b, chunk): out = softmax(q @ kv.T / sqrt(C)) @ kv
  output = w0 * image_features + w1 * fused,  w = softmax(modality_weights)

Sharding: the 32 independent (b, chunk) pairs are split 4-per-core across
8 NeuronCores (data parallel over batch x chunk; no communication).
Host prep (input/output layout only): shard slicing, bf16 copies, a
pre-packed KV (k, c)+ones array, Q in (q, c) layout, and the output is
returned from the device in (q, c) layout and transposed back on host.

Per-core device kernel, per (b, chunk) pair (all layouts partition-major):
  1. HWDGE loads: qb/kvb (bf16, (c, q) natural), kc (packed [KV | 1] in
     (k, c)), qt (fp32 Q in (q, c) for the final fuse).
  2. mm1: S^T tile (k=128p, 512q) = KVt.T @ Qt (bf16, fp32 PSUM accum).
  3. ACT exp with scale 1/sqrt(C): P^T panel in SBUF (bf16).
  4. mm2: (q=128p, 257f) = P^T.T @ [KV | 1]  -> unnormalized out | rowsum.
  5. DVE: g = (O' * recip(rowsum)) * w1 (bf16),
     out = (Qt * w0) + g (fp32, (q, c) layout), DMA out per panel.
No on-device transposes: the PE runs only the two matmul phases.
"""

import numpy as np

B, C, H, W = 4, 256, 128, 128
N = H * W
CHUNK = 2048
NCHUNKS = N // CHUNK         # 8
NCORES = 8
PAIRS = B * NCHUNKS          # 32
PPC = PAIRS // NCORES        # 4 pairs (chunks) per core
CT = C // 128                # 2 c-tiles
KT = CHUNK // 128            # 16 k-tiles
QT = CHUNK // 128            # 16 q-tiles
PAN = 512                    # q panel width
NPAN = CHUNK // PAN          # 4 panels
QT_PER_PAN = PAN // 128      # 4 q-tiles per panel
KCS = 272                    # kc tile stride (257 cols used)
SCALE = 1.0 / float(np.sqrt(C))

_BUILD_CACHE = {}


def _build(w0: float, w1: float):
    from contextlib import ExitStack

    import concourse.bass as bass
    import concourse.tile as tile
    from concourse import bacc, mybir

    f32 = mybir.dt.float32
    bf16 = mybir.dt.bfloat16
    Exp = mybir.ActivationFunctionType.Exp
    mult = mybir.AluOpType.mult
    add = mybir.AluOpType.add

    nc = bacc.Bacc("TRN2", target_bir_lowering=False, debug=False)
    qt_d = nc.dram_tensor("qt_sh", (PPC, CHUNK, C), f32, kind="ExternalInput")
    qb_d = nc.dram_tensor("qb_sh", (PPC, C, CHUNK), bf16, kind="ExternalInput")
    kvb_d = nc.dram_tensor("kvb_sh", (PPC, C, CHUNK), bf16, kind="ExternalInput")
    kc_d = nc.dram_tensor("kc_sh", (PPC, 128, KT * KCS), bf16, kind="ExternalInput")
    out_d = nc.dram_tensor("out_sh", (PPC, CHUNK, C), f32, kind="ExternalOutput")

    with ExitStack() as ctx:
        tc = ctx.enter_context(tile.TileContext(nc))
        po_qt = ctx.enter_context(tc.tile_pool(name="qt", bufs=2))
        po_qb = ctx.enter_context(tc.tile_pool(name="qb", bufs=2))
        po_kvb = ctx.enter_context(tc.tile_pool(name="kvb", bufs=2))
        po_kc = ctx.enter_context(tc.tile_pool(name="kc", bufs=2))
        po_pt = ctx.enter_context(tc.tile_pool(name="pt", bufs=2))
        po_out = ctx.enter_context(tc.tile_pool(name="outs", bufs=2))
        po_g = ctx.enter_context(tc.tile_pool(name="g", bufs=4))
        po_r = ctx.enter_context(tc.tile_pool(name="r", bufs=4))
        po_psS = ctx.enter_context(tc.tile_pool(name="psS", bufs=5, space="PSUM"))
        po_psO = ctx.enter_context(tc.tile_pool(name="psO", bufs=3, space="PSUM"))

        chunk_tiles = {}

        def emit_loads(p):
            qb = po_qb.tile([128, CT * CHUNK], bf16, name="qb")
            kvb = po_kvb.tile([128, CT * CHUNK], bf16, name="kvb")
            qt = po_qt.tile([128, QT * C], f32, name="qt")
            half = CHUNK // 2
            # first halves of kv + panel-0 of q land first so the PE can
            # start mm1 as early as possible
            for ci in range(CT):
                nc.sync.dma_start(
                    kvb[:, ci * CHUNK : ci * CHUNK + half],
                    kvb_d[p, ci * 128 : (ci + 1) * 128, 0:half],
                )
                # qb panel-0 issues from the other HWDGE engine in parallel
                nc.scalar.dma_start(
                    qb[:, ci * CHUNK : ci * CHUNK + PAN],
                    qb_d[p, ci * 128 : (ci + 1) * 128, 0:PAN],
                )
            for ci in range(CT):
                nc.sync.dma_start(
                    kvb[:, ci * CHUNK + half : (ci + 1) * CHUNK],
                    kvb_d[p, ci * 128 : (ci + 1) * 128, half:CHUNK],
                )
            for ci in range(CT):
                nc.sync.dma_start(
                    qb[:, ci * CHUNK + PAN : (ci + 1) * CHUNK],
                    qb_d[p, ci * 128 : (ci + 1) * 128, PAN:CHUNK],
                )
            kc = po_kc.tile([128, KT * KCS], bf16, name="kc")
            nc.sync.dma_start(kc[:], kc_d[p, :, :])
            # qt in (q, c): SBUF (128 part = q within tile, 16 q-tiles x C)
            qt3 = qt[:].rearrange("part (t c) -> part t c", c=C)
            qtd3 = qt_d[p].rearrange("(t part) c -> part t c", part=128)
            nc.sync.dma_start(qt3[:], qtd3[:])
            chunk_tiles[p] = (qb, kvb, qt, kc)

        emit_loads(0)

        for p in range(PPC):
            qb, kvb, qt, kc = chunk_tiles[p]
            if p + 1 < PPC:
                emit_loads(p + 1)

            outs = po_out.tile([128, QT * C], f32, name="outs")

            for pan in range(NPAN):
                # mm1 + exp -> P^T panel (k-tile major, 512 q cols each)
                pt = po_pt.tile([128, KT * PAN], bf16, name="pt")
                for j in range(KT):
                    psS = po_psS.tile([128, PAN], f32, name="psS")
                    for ci in range(CT):
                        nc.tensor.matmul(
                            psS[:],
                            lhsT=kvb[
                                :, ci * CHUNK + j * 128 : ci * CHUNK + (j + 1) * 128
                            ],
                            rhs=qb[
                                :,
                                ci * CHUNK + pan * PAN : ci * CHUNK + (pan + 1) * PAN,
                            ],
                            start=(ci == 0),
                            stop=(ci == CT - 1),
                        )
                    nc.scalar.activation(
                        pt[:, j * PAN : (j + 1) * PAN], psS[:], Exp, scale=SCALE
                    )

                # mm2 + normalize + fuse, all in (q, c) layout
                for tq in range(QT_PER_PAN):
                    t = pan * QT_PER_PAN + tq
                    psO = po_psO.tile([128, C + 1], f32, name="psO")
                    rot = (4 * (tq + 1)) % KT
                    jseq = [(rot + i) % KT for i in range(KT)]
                    for idx, j in enumerate(jseq):
                        nc.tensor.matmul(
                            psO[:],
                            lhsT=pt[:, j * PAN + tq * 128 : j * PAN + (tq + 1) * 128],
                            rhs=kc[:, j * KCS : j * KCS + C + 1],
                            start=(idx == 0),
                            stop=(idx == KT - 1),
                        )
                    r = po_r.tile([128, 1], f32, name="r")
                    nc.vector.reciprocal(r[:], psO[:, C : C + 1])
                    g = po_g.tile([128, C], bf16, name="g")
                    nc.vector.tensor_scalar(
                        g[:], psO[:, 0:C], r[:], float(w1), op0=mult, op1=mult
                    )
                    nc.vector.scalar_tensor_tensor(
                        outs[:, t * C : (t + 1) * C],
                        qt[:, t * C : (t + 1) * C],
                        float(w0),
                        g[:],
                        op0=mult,
                        op1=add,
                    )

                # store this panel's q-tiles (rows are (q, C) in DRAM);
                # on the very last panel store per q-tile to shorten the tail
                o3 = outs[:].rearrange("part (t c) -> part t c", c=C)
                od3 = out_d[p].rearrange("(t part) c -> part t c", part=128)
                t0 = pan * QT_PER_PAN
                if p == PPC - 1 and pan == NPAN - 1:
                    for dt in range(QT_PER_PAN):
                        nc.sync.dma_start(
                            od3[:, t0 + dt : t0 + dt + 1, :],
                            o3[:, t0 + dt : t0 + dt + 1, :],
                        )
                else:
                    nc.sync.dma_start(
                        od3[:, t0 : t0 + QT_PER_PAN, :], o3[:, t0 : t0 + QT_PER_PAN, :]
                    )

    nc.compile()
    return nc


def _get_nc(w0: float, w1: float):
    key = (round(float(w0), 9), round(float(w1), 9))
    if key not in _BUILD_CACHE:
        _BUILD_CACHE[key] = _build(*key)
    return _BUILD_CACHE[key]


def _pairs(arr: np.ndarray) -> np.ndarray:
    # (B, C, H, W) -> (PAIRS, C, CHUNK)
    return (
        arr.reshape(B, C, NCHUNKS, CHUNK)
        .transpose(0, 2, 1, 3)
        .reshape(PAIRS, C, CHUNK)
    )


def _unshard_qc(per_core: list[np.ndarray]) -> np.ndarray:
    # per-core (PPC, CHUNK, C) in (q, c) layout -> (B, C, H, W)
    pairs = np.concatenate(per_core, axis=0)  # (PAIRS, CHUNK, C)
    return np.ascontiguousarray(
        pairs.reshape(B, NCHUNKS, CHUNK, C)
        .transpose(0, 3, 1, 2)
        .reshape(B, C, H, W)
    )


def run(lidar_features, image_features, modality_weights, trace=False):
    import ml_dtypes

    from concourse import bass_utils

    mw = np.asarray(modality_weights, dtype=np.float64)
    e = np.exp(mw - mw.max())
    wsm = e / e.sum()
    w0, w1 = float(wsm[0]), float(wsm[1])

    nc = _get_nc(w0, w1)

    qp = _pairs(np.asarray(image_features, dtype=np.float32))
    kvp = _pairs(np.asarray(lidar_features, dtype=np.float32))
    qpb = qp.astype(ml_dtypes.bfloat16)
    kvpb = kvp.astype(ml_dtypes.bfloat16)
    # Q in (q, c) layout for the fp32 fuse term
    qpt = np.ascontiguousarray(qp.transpose(0, 2, 1))  # (PAIRS, CHUNK, C)
    # pre-packed KV (k, c) tiles + ones column, exactly the kc SBUF layout
    kcp = np.zeros((PAIRS, 128, KT, KCS), dtype=ml_dtypes.bfloat16)
    # kc[pair, k_in_tile, j, c] = KV[pair, c, j*128 + k_in_tile]
    kcp[:, :, :, 0:C] = kvpb.reshape(PAIRS, C, KT, 128).transpose(0, 3, 2, 1)
    kcp[:, :, :, C] = 1.0
    kcp = kcp.reshape(PAIRS, 128, KT * KCS)
    in_maps = [
        {
            "qt_sh": np.ascontiguousarray(qpt[i * PPC : (i + 1) * PPC]),
            "qb_sh": np.ascontiguousarray(qpb[i * PPC : (i + 1) * PPC]),
            "kvb_sh": np.ascontiguousarray(kvpb[i * PPC : (i + 1) * PPC]),
            "kc_sh": np.ascontiguousarray(kcp[i * PPC : (i + 1) * PPC]),
        }
        for i in range(NCORES)
    ]
    res = bass_utils.run_bass_kernel_spmd(
        nc, in_maps, core_ids=list(range(NCORES)), trace=trace
    )
    out = _unshard_qc([res.results[i]["out_sh"] for i in range(NCORES)])
    return out, res


def kernel(lidar_features, image_features, modality_weights) -> np.ndarray:
    out, _ = run(lidar_features, image_features, modality_weights, trace=False)
    return out



# revision 4
# speedup vs baseline: 1.4016x; 1.4016x over previous
"""Trainium2 Bass kernel for CrossAttentionFusion (fp8 DoubleRow version).

Reference computation (shapes hardcoded):
  B=4, C=256, H=W=128, N=16384, CHUNK=2048, nchunks=8.
  q  = image_features  reshaped to (B, nchunks, CHUNK, C)
  kv = lidar_features  reshaped to (B, nchunks, CHUNK, C)
  per (b, chunk): out = softmax(q @ kv.T / sqrt(C)) @ kv
  output = w0 * image_features + w1 * fused,  w = softmax(modality_weights)

Sharding: the 32 independent (b, chunk) pairs are split 4-per-core across
8 NeuronCores (data parallel over batch x chunk; no communication).

fp8 strategy: Q/KV are quantized to fp8e4 (e4m3, max 240) on host; both
matmuls run in MatmulPerfMode.DoubleRow (K=256 per instruction, 2 fp8
MACs/cell/cycle).  The attention probabilities are computed as
exp(s/sqrt(C) - 3) directly in fp8 by the ACT engine; the -3 bias keeps
max P ~17 < 240 (fp8e4 max) and cancels in the softmax ratio because the
row-sum (via the ones column of kc) is scaled identically.

Per-core device kernel, per (b, chunk) pair (all layouts partition-major):
  1. HWDGE loads: qb/kvb (fp8, [128, ct, k] c-major), kc (packed [KV | 1]
     fp8 in [128, kt, 257]), qt (fp32 Q in (q, c) for the final fuse).
  2. mm1 DoubleRow: S^T tile (k=128p, 512q) = KV.T @ Q, K=256 per instr.
  3. ACT exp (scale 1/sqrt(C), bias -3): P^T panel in SBUF (fp8).
  4. mm2 DoubleRow: (q=128p, 257f) = P^T.T @ [KV | 1] over 8 j-pairs.
  5. DVE: g = (O' * recip(rowsum)) * w1 (bf16),
     out = (Qt * w0) + g (fp32, (q, c) layout), DMA out per panel.
"""

import numpy as np

B, C, H, W = 4, 256, 128, 128
N = H * W
CHUNK = 2048
NCHUNKS = N // CHUNK         # 8
NCORES = 8
PAIRS = B * NCHUNKS          # 32
PPC = PAIRS // NCORES        # 4 pairs (chunks) per core
CT = C // 128                # 2 c-tiles
KT = CHUNK // 128            # 16 k-tiles
QT = CHUNK // 128            # 16 q-tiles
PAN = 512                    # q panel width
NPAN = CHUNK // PAN          # 4 panels
QT_PER_PAN = PAN // 128      # 4 q-tiles per panel
KCS = 272                    # kc tile stride (257 cols used)
JP = KT // 2                 # 8 j-pairs for DoubleRow mm2
SCALE = 1.0 / float(np.sqrt(C))
EBIAS = -3.0                 # exp bias: keeps max P < fp8e4 max (240)

_BUILD_CACHE = {}


def _build(w0: float, w1: float):
    from contextlib import ExitStack

    import concourse.bass as bass
    import concourse.tile as tile
    from concourse import bacc, mybir

    f32 = mybir.dt.float32
    bf16 = mybir.dt.bfloat16
    f8 = mybir.dt.float8e4
    DR = mybir.MatmulPerfMode.DoubleRow
    Exp = mybir.ActivationFunctionType.Exp
    mult = mybir.AluOpType.mult
    add = mybir.AluOpType.add

    nc = bacc.Bacc("TRN2", target_bir_lowering=False, debug=False)
    qt_d = nc.dram_tensor("qt_sh", (PPC, CHUNK, C), f32, kind="ExternalInput")
    qb_d = nc.dram_tensor("qb_sh", (PPC, C, CHUNK), f8, kind="ExternalInput")
    kvb_d = nc.dram_tensor("kvb_sh", (PPC, C, CHUNK), f8, kind="ExternalInput")
    kc_d = nc.dram_tensor("kc_sh", (PPC, 128, KT * KCS), f8, kind="ExternalInput")
    out_d = nc.dram_tensor("out_sh", (PPC, CHUNK, C), f32, kind="ExternalOutput")

    with ExitStack() as ctx:
        tc = ctx.enter_context(tile.TileContext(nc))
        po_qt = ctx.enter_context(tc.tile_pool(name="qt", bufs=2))
        po_qb = ctx.enter_context(tc.tile_pool(name="qb", bufs=2))
        po_kvb = ctx.enter_context(tc.tile_pool(name="kvb", bufs=2))
        po_kc = ctx.enter_context(tc.tile_pool(name="kc", bufs=2))
        po_pt = ctx.enter_context(tc.tile_pool(name="pt", bufs=2))
        po_out = ctx.enter_context(tc.tile_pool(name="outs", bufs=2))
        po_g = ctx.enter_context(tc.tile_pool(name="g", bufs=4))
        po_r = ctx.enter_context(tc.tile_pool(name="r", bufs=4))
        po_psS = ctx.enter_context(tc.tile_pool(name="psS", bufs=5, space="PSUM"))
        po_psO = ctx.enter_context(tc.tile_pool(name="psO", bufs=3, space="PSUM"))
        po_const = ctx.enter_context(tc.tile_pool(name="consts", bufs=1))

        ebias = po_const.tile([128, 1], f32, name="ebias")
        nc.gpsimd.memset(ebias[:], EBIAS)

        chunk_tiles = {}

        def emit_loads(p):
            qb = po_qb.tile([128, CT, CHUNK], f8, name="qb")
            kvb = po_kvb.tile([128, CT, CHUNK], f8, name="kvb")
            qt = po_qt.tile([128, QT * C], f32, name="qt")
            half = CHUNK // 2
            # first halves of kv + panel-0 of q land first so the PE can
            # start mm1 as early as possible
            for ci in range(CT):
                nc.sync.dma_start(
                    kvb[:, ci : ci + 1, 0:half],
                    kvb_d[p, ci * 128 : (ci + 1) * 128, 0:half],
                )
                # qb panel-0 issues from the other HWDGE engine in parallel
                nc.scalar.dma_start(
                    qb[:, ci : ci + 1, 0:PAN],
                    qb_d[p, ci * 128 : (ci + 1) * 128, 0:PAN],
                )
            for ci in range(CT):
                nc.sync.dma_start(
                    kvb[:, ci : ci + 1, half:CHUNK],
                    kvb_d[p, ci * 128 : (ci + 1) * 128, half:CHUNK],
                )
            for ci in range(CT):
                nc.sync.dma_start(
                    qb[:, ci : ci + 1, PAN:CHUNK],
                    qb_d[p, ci * 128 : (ci + 1) * 128, PAN:CHUNK],
                )
            kc = po_kc.tile([128, KT, KCS], f8, name="kc")
            nc.sync.dma_start(
                kc[:].rearrange("part a b -> part (a b)"), kc_d[p, :, :]
            )
            # qt in (q, c): SBUF (128 part = q within tile, 16 q-tiles x C)
            qt3 = qt[:].rearrange("part (t c) -> part t c", c=C)
            qtd3 = qt_d[p].rearrange("(t part) c -> part t c", part=128)
            nc.sync.dma_start(qt3[:], qtd3[:])
            chunk_tiles[p] = (qb, kvb, qt, kc)

        emit_loads(0)

        for p in range(PPC):
            qb, kvb, qt, kc = chunk_tiles[p]
            if p + 1 < PPC:
                emit_loads(p + 1)

            outs = po_out.tile([128, QT * C], f32, name="outs")

            for pan in range(NPAN):
                # mm1 DoubleRow + exp -> P^T panel (k-tile major, fp8)
                pt = po_pt.tile([128, KT, PAN], f8, name="pt")
                for j in range(KT):
                    psS = po_psS.tile([128, PAN], f32, name="psS")
                    nc.tensor.matmul(
                        psS[:],
                        lhsT=kvb[:, :, j * 128 : (j + 1) * 128],
                        rhs=qb[:, :, pan * PAN : (pan + 1) * PAN],
                        start=True,
                        stop=True,
                        perf_mode=DR,
                    )
                    nc.scalar.activation(
                        pt[:, j : j + 1, :], psS[:], Exp, bias=ebias[:], scale=SCALE
                    )

                # mm2 DoubleRow + normalize + fuse, all in (q, c) layout
                for tq in range(QT_PER_PAN):
                    t = pan * QT_PER_PAN + tq
                    psO = po_psO.tile([128, C + 1], f32, name="psO")
                    rot = (2 * (tq + 1)) % JP
                    jseq = [(rot + i) % JP for i in range(JP)]
                    for idx, jp in enumerate(jseq):
                        nc.tensor.matmul(
                            psO[:],
                            lhsT=pt[:, 2 * jp : 2 * jp + 2, tq * 128 : (tq + 1) * 128],
                            rhs=kc[:, 2 * jp : 2 * jp + 2, 0 : C + 1],
                            start=(idx == 0),
                            stop=(idx == JP - 1),
                            perf_mode=DR,
                        )
                    r = po_r.tile([128, 1], f32, name="r")
                    nc.vector.reciprocal(r[:], psO[:, C : C + 1])
                    g = po_g.tile([128, C], bf16, name="g")
                    nc.vector.tensor_scalar(
                        g[:], psO[:, 0:C], r[:], float(w1), op0=mult, op1=mult
                    )
                    nc.vector.scalar_tensor_tensor(
                        outs[:, t * C : (t + 1) * C],
                        qt[:, t * C : (t + 1) * C],
                        float(w0),
                        g[:],
                        op0=mult,
                        op1=add,
                    )

                # store this panel's q-tiles (rows are (q, C) in DRAM);
                # on the very last panel store per q-tile to shorten the tail
                o3 = outs[:].rearrange("part (t c) -> part t c", c=C)
                od3 = out_d[p].rearrange("(t part) c -> part t c", part=128)
                t0 = pan * QT_PER_PAN
                if p == PPC - 1 and pan == NPAN - 1:
                    for dt in range(QT_PER_PAN):
                        nc.sync.dma_start(
                            od3[:, t0 + dt : t0 + dt + 1, :],
                            o3[:, t0 + dt : t0 + dt + 1, :],
                        )
                else:
                    nc.sync.dma_start(
                        od3[:, t0 : t0 + QT_PER_PAN, :], o3[:, t0 : t0 + QT_PER_PAN, :]
                    )

    nc.compile()
    return nc


def _get_nc(w0: float, w1: float):
    key = (round(float(w0), 9), round(float(w1), 9))
    if key not in _BUILD_CACHE:
        _BUILD_CACHE[key] = _build(*key)
    return _BUILD_CACHE[key]


def _pairs(arr: np.ndarray) -> np.ndarray:
    # (B, C, H, W) -> (PAIRS, C, CHUNK)
    return (
        arr.reshape(B, C, NCHUNKS, CHUNK)
        .transpose(0, 2, 1, 3)
        .reshape(PAIRS, C, CHUNK)
    )


def _unshard_qc(per_core: list[np.ndarray]) -> np.ndarray:
    # per-core (PPC, CHUNK, C) in (q, c) layout -> (B, C, H, W)
    pairs = np.concatenate(per_core, axis=0)  # (PAIRS, CHUNK, C)
    return np.ascontiguousarray(
        pairs.reshape(B, NCHUNKS, CHUNK, C)
        .transpose(0, 3, 1, 2)
        .reshape(B, C, H, W)
    )


def run(lidar_features, image_features, modality_weights, trace=False):
    import ml_dtypes

    from concourse import bass_utils

    f8 = ml_dtypes.float8_e4m3

    mw = np.asarray(modality_weights, dtype=np.float64)
    e = np.exp(mw - mw.max())
    wsm = e / e.sum()
    w0, w1 = float(wsm[0]), float(wsm[1])

    nc = _get_nc(w0, w1)

    qp = _pairs(np.asarray(image_features, dtype=np.float32))
    kvp = _pairs(np.asarray(lidar_features, dtype=np.float32))
    qpb = qp.astype(f8)
    kvpb = kvp.astype(f8)
    # Q in (q, c) layout for the fp32 fuse term
    qpt = np.ascontiguousarray(qp.transpose(0, 2, 1))  # (PAIRS, CHUNK, C)
    # pre-packed KV (k, c) tiles + ones column, exactly the kc SBUF layout
    kcp = np.zeros((PAIRS, 128, KT, KCS), dtype=f8)
    # kc[pair, k_in_tile, j, c] = KV[pair, c, j*128 + k_in_tile]
    kcp[:, :, :, 0:C] = kvpb.reshape(PAIRS, C, KT, 128).transpose(0, 3, 2, 1)
    kcp[:, :, :, C] = 1.0
    kcp = kcp.reshape(PAIRS, 128, KT * KCS)
    in_maps = [
        {
            "qt_sh": np.ascontiguousarray(qpt[i * PPC : (i + 1) * PPC]),
            "qb_sh": np.ascontiguousarray(qpb[i * PPC : (i + 1) * PPC]),
            "kvb_sh": np.ascontiguousarray(kvpb[i * PPC : (i + 1) * PPC]),
            "kc_sh": np.ascontiguousarray(kcp[i * PPC : (i + 1) * PPC]),
        }
        for i in range(NCORES)
    ]
    res = bass_utils.run_bass_kernel_spmd(
        nc, in_maps, core_ids=list(range(NCORES)), trace=trace
    )
    out = _unshard_qc([res.results[i]["out_sh"] for i in range(NCORES)])
    return out, res


def kernel(lidar_features, image_features, modality_weights) -> np.ndarray:
    out, _ = run(lidar_features, image_features, modality_weights, trace=False)
    return out
